# revision 23
# baseline (speedup 1.0000x reference)
"""MultiHeadAttention Trainium2 kernel, 8-way tensor-parallel over heads.

B=4, T=2048, C=1024, H=16 heads, Dh=64. Each of the 8 NeuronCores owns 2
heads. All matmuls in bf16 (fp32 PSUM accumulation).

Schedule: software-pipelined across batches. The attention phase for batch b
is Activation-engine-bound (exp of S^T); the PE stall slots inside it are
filled by injecting the QKV projection of batch b+1 and the output projection
of batch b-1, unit by unit, so the PE stream never idles (keeps the HAM clock
gate at 2.4 GHz).

Per batch:
  - QKV: q^T/k^T as [2*Dh, T] (PSUM->SBUF bias-add on DVE), V via PE
    transpose into v1 k-tiles [V_h0|1|junk|1|junk|V_h1] (ones column yields
    the softmax denominator for free during PV).
  - attention per 512-query tile: S^T matmuls (contraction Dh) write a
    single 4-bank PSUM tile holding a PAIR of k-tiles; one exp (ScalarE)
    per pair halves activation call overhead; PV accumulates [P@V | Z] in
    two PSUM banks; normalization reads PSUM directly (reciprocal on DVE,
    partition-broadcast on GpSimd, multiply on DVE).
  - out-projection: partial against this core's 128 W_out columns,
    PSUM->SBUF copy on DVE, DMA to HBM. Output bias is added on the host
    after the 8-core partial sum.
"""
import sys
sys.path.insert(0, '/opt/trn_rl_repo')
import numpy as np

import concourse.bass as bass
import concourse.mybir as mybir
import concourse.tile as tile
from concourse import bacc
from concourse.bass_utils import run_bass_kernel_spmd
from concourse.masks import make_identity

F32 = mybir.dt.float32
BF16 = mybir.dt.bfloat16
AF = mybir.ActivationFunctionType

B, T, C = 4, 2048, 1024
H, DH = 16, 64
NCORES = 8
HPC = H // NCORES          # heads per core (2)
D2 = HPC * DH              # 128, local concat dim
BT = B * T                 # 8192
NT = T // 512              # q/t tiles of 512 per batch (4)
NK = T // 128              # k tiles of 128 per batch (16)
NP = NK // 2               # k-tile pairs (8)
CCH = C // 128             # contraction chunks (8)

_NC_CACHE = {}


def build_nc():
    nc = bacc.Bacc()

    xp = nc.dram_tensor("xp", [128, B * NT, CCH, 512], BF16, kind="ExternalInput")
    wq = nc.dram_tensor("wq", [128, CCH, D2], BF16, kind="ExternalInput")
    wk = nc.dram_tensor("wk", [128, CCH, D2], BF16, kind="ExternalInput")
    wv = nc.dram_tensor("wv", [128, CCH, D2], BF16, kind="ExternalInput")
    bq = nc.dram_tensor("bq", [D2, 1], F32, kind="ExternalInput")
    bk = nc.dram_tensor("bk", [D2, 1], F32, kind="ExternalInput")
    bv = nc.dram_tensor("bv", [D2, 1], F32, kind="ExternalInput")
    wo = nc.dram_tensor("wo", [128, C], BF16, kind="ExternalInput")
    y = nc.dram_tensor("y", [BT, C], F32, kind="ExternalOutput")

    with tile.TileContext(nc) as tc:
        with (
            tc.tile_pool(name="singles", bufs=1) as singles,
            tc.tile_pool(name="xin", bufs=4) as xin,
            tc.tile_pool(name="qkv", bufs=3) as qkv,
            tc.tile_pool(name="vtmp", bufs=2) as vtmp_pool,
            tc.tile_pool(name="esb", bufs=4) as esb,
            tc.tile_pool(name="rsb", bufs=2) as rsb,
            tc.tile_pool(name="osb", bufs=2) as osb,
            tc.tile_pool(name="outsb", bufs=3) as outsb,
            # 8 PSUM banks: s2 1x4 + pv 2x1 + aux 2x1
            tc.tile_pool(name="s2_ps", bufs=1, space="PSUM") as s2_ps,
            tc.tile_pool(name="pv_ps", bufs=2, space="PSUM") as pv_ps,
            tc.tile_pool(name="aux_ps", bufs=2, space="PSUM") as aux_ps,
        ):
            ident = singles.tile([128, 128], F32)
            make_identity(nc, ident)
            warm_r = singles.tile([128, 512], BF16, tag="warm_r")
            nc.vector.memset(warm_r, 1.0)
            for wi in range(18):
                wps = aux_ps.tile([128, 512], F32, tag="aux", name=f"warm{wi}")
                nc.tensor.matmul(out=wps, lhsT=warm_r[:, 0:128], rhs=warm_r,
                                 start=True, stop=True)
            ones16 = singles.tile([128, NK, 1], BF16)
            nc.vector.memset(ones16, 1.0)

            wq_sb = singles.tile([128, CCH, D2], BF16, tag="wq")
            wk_sb = singles.tile([128, CCH, D2], BF16, tag="wk")
            wv_sb = singles.tile([128, CCH, D2], BF16, tag="wv")
            for w_dram, w_sb in ((wq, wq_sb), (wk, wk_sb), (wv, wv_sb)):
                nc.sync.dma_start(out=w_sb, in_=w_dram[:, :, :])
            bq_sb = singles.tile([D2, 1], F32, tag="bq")
            bk_sb = singles.tile([D2, 1], F32, tag="bk")
            bv_sb = singles.tile([D2, 1], F32, tag="bv")
            nc.sync.dma_start(out=bq_sb, in_=bq[:, :])
            nc.sync.dma_start(out=bk_sb, in_=bk[:, :])
            nc.sync.dma_start(out=bv_sb, in_=bv[:, :])
            wo_sb = singles.tile([128, C], BF16, tag="wo")
            nc.sync.dma_start(out=wo_sb, in_=wo[:, :])

            qkv_tiles = {}

            def gen_qkv(b):
                """QKV projection units for batch b. Yields at ~2-matmul
                granularity so the injector can spread them through the
                attention phase of batch b-1."""
                qT = qkv.tile([D2, T], BF16, tag="q", name=f"q{b}")
                kT = qkv.tile([D2, T], BF16, tag="k", name=f"k{b}")
                v1 = qkv.tile([128, NK, 193], BF16, tag="v", name=f"v{b}")
                qkv_tiles[b] = (qT, kT, v1)
                xts = []
                for tt in range(3):
                    xt = xin.tile([128, CCH, 512], BF16, tag="x",
                                  name=f"x{b}_{tt}")
                    nc.sync.dma_start(out=xt, in_=xp[:, b * NT + tt, :, :])
                    xts.append(xt)
                if b < 3:
                    # the junk columns [65:129) are consumed as stationary
                    # weights by the PV matmuls (their PSUM rows are never
                    # read) — zero them once per buffer so leftover SBUF bits
                    # can't be NaN/Inf patterns (v1 pool has 3 buffers and
                    # later writes never touch the junk range).
                    nc.gpsimd.memset(v1[:, :, 65:129], 0.0)
                nc.vector.tensor_copy(out=v1[:, :, DH:DH + 1], in_=ones16)
                nc.vector.tensor_copy(out=v1[:, :, 97:98], in_=ones16)
                yield
                for tt in range(NT):
                    if tt + 3 < NT:
                        xt = xin.tile([128, CCH, 512], BF16, tag="x",
                                      name=f"x{b}_{tt + 3}")
                        nc.sync.dma_start(
                            out=xt, in_=xp[:, b * NT + tt + 3, :, :])
                        xts.append(xt)
                    t0 = tt * 512
                    xt = xts[tt]
                    for w_sb, b_sb, dest in ((wq_sb, bq_sb, qT),
                                             (wk_sb, bk_sb, kT)):
                        ps = aux_ps.tile([128, 512], F32, tag="aux")
                        for ci in range(CCH):
                            nc.tensor.matmul(out=ps, lhsT=w_sb[:, ci, :],
                                             rhs=xt[:, ci, :],
                                             start=(ci == 0),
                                             stop=(ci == CCH - 1))
                            if ci % 2 == 1:
                                yield
                        nc.vector.tensor_scalar_add(out=dest[:, t0:t0 + 512],
                                                    in0=ps, scalar1=b_sb)
                        yield
                    ps = aux_ps.tile([128, 512], F32, tag="aux")
                    for ci in range(CCH):
                        nc.tensor.matmul(out=ps, lhsT=wv_sb[:, ci, :],
                                         rhs=xt[:, ci, :],
                                         start=(ci == 0), stop=(ci == CCH - 1))
                        if ci % 2 == 1:
                            yield
                    vt = vtmp_pool.tile([128, 512], F32)
                    nc.vector.tensor_scalar_add(out=vt, in0=ps, scalar1=bv_sb)
                    yield
                    for s in range(4):
                        tp = aux_ps.tile([128, 512], F32, tag="aux")
                        nc.tensor.transpose(out=tp[:, 0:128],
                                            in_=vt[:, s * 128:(s + 1) * 128],
                                            identity=ident)
                        kt = tt * 4 + s
                        sl = v1[:, kt, :]
                        dst = bass.AP(tensor=sl.tensor, offset=sl.offset,
                                      ap=[list(sl.ap[0]), [129, 2], [1, DH]])
                        nc.vector.tensor_copy(
                            out=dst,
                            in_=tp[:, 0:128].rearrange("p (g x) -> p g x", g=2))
                        yield

            def gen_outproj(b, oT2, ts_lo, ts_hi):
                """Output-projection units for batch b, t-tiles [ts_lo, ts_hi)
                (needs the corresponding oT2 columns written)."""
                for ts in range(ts_lo, ts_hi):
                    ot = outsb.tile([128, C], F32)
                    for n in range(2):
                        n0 = n * 512
                        ops = aux_ps.tile([128, 512], F32, tag="aux")
                        nc.tensor.matmul(
                            out=ops,
                            lhsT=oT2[:, ts * 128:(ts + 1) * 128],
                            rhs=wo_sb[:, n0:n0 + 512],
                            start=True, stop=True)
                        nc.vector.tensor_copy(out=ot[:, n0:n0 + 512], in_=ops)
                        yield
                    nc.sync.dma_start(
                        out=y[b * T + ts * 128:b * T + (ts + 1) * 128, :],
                        in_=ot)

            class Injector:
                """Two queues: `pri` (QKV of the next batch — must fully issue
                before that batch's attention starts) and `bg` (out-projection
                chunks — can spill across batch boundaries)."""

                def __init__(self):
                    self.pri = []
                    self.bg = []

                def add(self, g):
                    self.pri.append(g)

                def add_bg(self, g):
                    self.bg.append(g)

                def step(self, n):
                    while n > 0:
                        q = self.pri if self.pri else self.bg
                        if not q:
                            return
                        try:
                            next(q[0])
                            n -= 1
                        except StopIteration:
                            q.pop(0)

                def drain_pri(self):
                    while self.pri:
                        try:
                            next(self.pri[0])
                        except StopIteration:
                            self.pri.pop(0)

                def drain_all(self):
                    self.drain_pri()
                    while self.bg:
                        try:
                            next(self.bg[0])
                        except StopIteration:
                            self.bg.pop(0)

            def attention(b, inj):
                qT, kT, v1 = qkv_tiles[b]
                oT2 = osb.tile([128, T], BF16, tag="o2", name=f"o2_{b}")
                lh = [(0, DH + 1), (DH + 1, 193)]
                for qt in range(NT):
                    q0 = qt * 512
                    pv0 = pv_ps.tile([128, 512], F32, tag="pv")
                    pv1 = pv_ps.tile([128, 512], F32, tag="pv")
                    pvs = [pv0[0:DH + 1, :], pv1[:, :]]
                    s2 = s2_ps.tile([128, 2, 1024], F32, tag="s2")
                    ets = []
                    for p in range(NP):
                        for j in range(2):
                            kt = 2 * p + j
                            for h in range(HPC):
                                hs = h * DH
                                nc.tensor.matmul(
                                    out=s2[:, j, h * 512:(h + 1) * 512],
                                    lhsT=kT[hs:hs + DH,
                                            kt * 128:(kt + 1) * 128],
                                    rhs=qT[hs:hs + DH, q0:q0 + 512],
                                    start=True, stop=True)
                        if p >= 1:
                            e = ets[p - 1]
                            for j in range(2):
                                kt = 2 * (p - 1) + j
                                for h in range(HPC):
                                    nc.tensor.matmul(
                                        out=pvs[h],
                                        lhsT=v1[:, kt, lh[h][0]:lh[h][1]],
                                        rhs=e[:, j, h * 512:(h + 1) * 512],
                                        start=(kt == 0), stop=False)
                        et = esb.tile([128, 2, 1024], BF16)
                        nc.scalar.activation(out=et, in_=s2,
                                             func=AF.Exp, scale=0.125)
                        ets.append(et)
                        inj.step(3)
                    e = ets[NP - 1]
                    for j in range(2):
                        kt = NK - 2 + j
                        for h in range(HPC):
                            nc.tensor.matmul(
                                out=pvs[h],
                                lhsT=v1[:, kt, lh[h][0]:lh[h][1]],
                                rhs=e[:, j, h * 512:(h + 1) * 512],
                                start=False, stop=(kt == NK - 1))
                    # evacuate each PV accumulator with one SBUF copy (frees
                    # the PSUM bank for the next qt immediately), then
                    # normalize from the copy.
                    # h0: num@[0:64], Z@[64]; h1: num@[64:128], Z@[32]
                    for h, pv, zrow, o_lo, o_hi in (
                            (0, pv0, DH, 0, DH),
                            (1, pv1, 32, DH, 128)):
                        z = rsb.tile([1, 512], F32, tag="z")
                        nc.vector.tensor_copy(out=z, in_=pv[zrow:zrow + 1, :])
                        r = rsb.tile([1, 512], F32, tag="r")
                        nc.vector.reciprocal_approx_fast(out=r, in_=z)
                        rbc = rsb.tile([128, 512], F32, tag="rbc")
                        nc.gpsimd.partition_broadcast(rbc[0:o_hi, :], r)
                        nc.vector.tensor_mul(out=oT2[o_lo:o_hi, q0:q0 + 512],
                                             in0=pv[o_lo:o_hi, :],
                                             in1=rbc[o_lo:o_hi, :])
                    # this qt's out-projection columns are now computable
                    inj.add_bg(gen_outproj(b, oT2, qt * 4, qt * 4 + 4))
                    inj.step(2)
                return oT2

            # ---- pipelined schedule ----
            inj = Injector()
            inj.add(gen_qkv(0))
            inj.drain_pri()
            for b in range(B):
                if b + 1 < B:
                    inj.add(gen_qkv(b + 1))
                attention(b, inj)
                inj.drain_pri()
            inj.drain_all()

    nc.compile()
    return nc


def make_in_maps(x, W_qkv, b_qkv, W_out, b_out):
    import ml_dtypes
    bf = ml_dtypes.bfloat16
    # x pre-tiled to the exact SBUF layout: xp[p, tile, ci, c] = x[tile*512+c, ci*128+p]
    xp = np.ascontiguousarray(
        x.reshape(B * NT, 512, CCH, 128).transpose(3, 0, 2, 1).astype(bf))
    in_maps = []
    for c in range(NCORES):
        r0 = c * D2
        def wshuf(wslice):
            # [D2, C] weight rows -> lhsT chunks [128 p, CCH, D2]
            return np.ascontiguousarray(
                wslice.T.reshape(CCH, 128, D2).transpose(1, 0, 2).astype(bf))
        wqc = wshuf(W_qkv[r0:r0 + D2, :])
        wkc = wshuf(W_qkv[C + r0:C + r0 + D2, :])
        wvc = wshuf(W_qkv[2 * C + r0:2 * C + r0 + D2, :])
        bqc = np.ascontiguousarray(b_qkv[r0:r0 + D2].reshape(D2, 1))
        bkc = np.ascontiguousarray(b_qkv[C + r0:C + r0 + D2].reshape(D2, 1))
        bvc = np.ascontiguousarray(b_qkv[2 * C + r0:2 * C + r0 + D2].reshape(D2, 1))
        woc = np.ascontiguousarray(W_out[:, r0:r0 + D2].T.astype(bf))
        in_maps.append({
            "xp": xp, "wq": wqc, "wk": wkc, "wv": wvc,
            "bq": bqc, "bk": bkc, "bv": bvc, "wo": woc,
        })
    return in_maps


def run(x, W_qkv, b_qkv, W_out, b_out, trace=False):
    if "nc" not in _NC_CACHE:
        _NC_CACHE["nc"] = build_nc()
    nc = _NC_CACHE["nc"]
    in_maps = make_in_maps(
        np.asarray(x, dtype=np.float32), np.asarray(W_qkv, dtype=np.float32),
        np.asarray(b_qkv, dtype=np.float32), np.asarray(W_out, dtype=np.float32),
        np.asarray(b_out, dtype=np.float32))
    res = run_bass_kernel_spmd(nc, in_maps, core_ids=list(range(NCORES)),
                               trace=trace)
    acc = np.zeros((BT, C), dtype=np.float64)
    for c in range(NCORES):
        acc += res.results[c]["y"]
    acc += np.asarray(b_out, dtype=np.float64)[None, :]
    out = acc.astype(np.float32).reshape(B, T, C)
    return out, res


def kernel(x, W_qkv, b_qkv, W_out, b_out):
    out, _ = run(x, W_qkv, b_qkv, W_out, b_out, trace=False)
    return out


# revision 25
# speedup vs baseline: 1.1052x; 1.1052x over previous
"""MultiHeadAttention Trainium2 kernel, 8-way tensor-parallel over heads.

B=4, T=2048, C=1024, H=16 heads, Dh=64. Each of the 8 NeuronCores owns 2
heads. All matmuls in bf16 (fp32 PSUM accumulation).

Schedule: software-pipelined across batches. The attention phase for batch b
is Activation-engine-bound (exp of S^T); the PE stall slots inside it are
filled by injecting the QKV projection of batch b+1 and the output projection
of batch b-1, unit by unit, so the PE stream never idles (keeps the HAM clock
gate at 2.4 GHz).

Per batch:
  - QKV: q^T/k^T as [2*Dh, T] (PSUM->SBUF bias-add on DVE), V via PE
    transpose into v1 k-tiles [V_h0|1|junk|1|junk|V_h1] (ones column yields
    the softmax denominator for free during PV).
  - attention per 512-query tile: S^T matmuls (contraction Dh) write a
    single 4-bank PSUM tile holding a PAIR of k-tiles; one exp (ScalarE)
    per pair halves activation call overhead; PV accumulates [P@V | Z] in
    two PSUM banks; normalization reads PSUM directly (reciprocal on DVE,
    partition-broadcast on GpSimd, multiply on DVE).
  - out-projection: partial against this core's 128 W_out columns,
    PSUM->SBUF copy on DVE, DMA to HBM. Output bias is added on the host
    after the 8-core partial sum.
"""
import sys
sys.path.insert(0, '/opt/trn_rl_repo')
import numpy as np

import concourse.bass as bass
import concourse.mybir as mybir
import concourse.tile as tile
from concourse import bacc
from concourse.bass_utils import run_bass_kernel_spmd
from concourse.masks import make_identity

F32 = mybir.dt.float32
BF16 = mybir.dt.bfloat16
AF = mybir.ActivationFunctionType

B, T, C = 4, 2048, 1024
H, DH = 16, 64
NCORES = 8
HPC = H // NCORES          # heads per core (2)
D2 = HPC * DH              # 128, local concat dim
BT = B * T                 # 8192
NT = T // 512              # q/t tiles of 512 per batch (4)
NK = T // 128              # k tiles of 128 per batch (16)
NP = NK // 2               # k-tile pairs (8)
CCH = C // 128             # contraction chunks (8)

_NC_CACHE = {}


def build_nc():
    nc = bacc.Bacc()

    xp = nc.dram_tensor("xp", [128, B * NT, CCH, 512], BF16, kind="ExternalInput")
    wq = nc.dram_tensor("wq", [128, CCH, D2], BF16, kind="ExternalInput")
    wk = nc.dram_tensor("wk", [128, CCH, D2], BF16, kind="ExternalInput")
    wv = nc.dram_tensor("wv", [128, CCH, D2], BF16, kind="ExternalInput")
    bq = nc.dram_tensor("bq", [D2, 1], F32, kind="ExternalInput")
    bk = nc.dram_tensor("bk", [D2, 1], F32, kind="ExternalInput")
    bv = nc.dram_tensor("bv", [D2, 1], F32, kind="ExternalInput")
    wo = nc.dram_tensor("wo", [128, C], BF16, kind="ExternalInput")
    y = nc.dram_tensor("y", [BT, C], F32, kind="ExternalOutput")

    with tile.TileContext(nc) as tc:
        with (
            tc.tile_pool(name="singles", bufs=1) as singles,
            tc.tile_pool(name="xin", bufs=4) as xin,
            tc.tile_pool(name="qkv", bufs=3) as qkv,
            tc.tile_pool(name="vtmp", bufs=2) as vtmp_pool,
            tc.tile_pool(name="esb", bufs=4) as esb,
            tc.tile_pool(name="rsb", bufs=2) as rsb,
            tc.tile_pool(name="osb", bufs=2) as osb,
            tc.tile_pool(name="outsb", bufs=3) as outsb,
            # 8 PSUM banks: s2 2x2 + pv 2x1 + aux 2x1
            tc.tile_pool(name="s2_ps", bufs=2, space="PSUM") as s2_ps,
            tc.tile_pool(name="pv_ps", bufs=2, space="PSUM") as pv_ps,
            tc.tile_pool(name="aux_ps", bufs=2, space="PSUM") as aux_ps,
        ):
            ident = singles.tile([128, 128], F32)
            make_identity(nc, ident)
            warm_r = singles.tile([128, 512], BF16, tag="warm_r")
            nc.vector.memset(warm_r, 1.0)
            for wi in range(18):
                wps = aux_ps.tile([128, 512], F32, tag="aux", name=f"warm{wi}")
                nc.tensor.matmul(out=wps, lhsT=warm_r[:, 0:128], rhs=warm_r,
                                 start=True, stop=True)
            ones16 = singles.tile([128, NK, 1], BF16)
            nc.vector.memset(ones16, 1.0)

            wq_sb = singles.tile([128, CCH, D2], BF16, tag="wq")
            wk_sb = singles.tile([128, CCH, D2], BF16, tag="wk")
            wv_sb = singles.tile([128, CCH, D2], BF16, tag="wv")
            bq_sb = singles.tile([D2, 1], F32, tag="bq")
            bk_sb = singles.tile([D2, 1], F32, tag="bk")
            bv_sb = singles.tile([D2, 1], F32, tag="bv")
            wo_sb = singles.tile([128, C], BF16, tag="wo")
            nc.sync.dma_start(out=wq_sb, in_=wq[:, :, :])
            nc.sync.dma_start(out=bq_sb, in_=bq[:, :])

            def load_rest_of_weights():
                nc.sync.dma_start(out=wk_sb, in_=wk[:, :, :])
                nc.sync.dma_start(out=wv_sb, in_=wv[:, :, :])
                nc.sync.dma_start(out=bk_sb, in_=bk[:, :])
                nc.sync.dma_start(out=bv_sb, in_=bv[:, :])
                nc.sync.dma_start(out=wo_sb, in_=wo[:, :])

            qkv_tiles = {}

            def gen_qkv(b):
                """QKV projection units for batch b. Yields at ~2-matmul
                granularity so the injector can spread them through the
                attention phase of batch b-1."""
                qT = qkv.tile([D2, T], BF16, tag="q", name=f"q{b}")
                kT = qkv.tile([D2, T], BF16, tag="k", name=f"k{b}")
                v1 = qkv.tile([128, NK, 193], BF16, tag="v", name=f"v{b}")
                qkv_tiles[b] = (qT, kT, v1)
                xts = []
                xt = xin.tile([128, CCH, 512], BF16, tag="x", name=f"x{b}_0")
                nc.sync.dma_start(out=xt, in_=xp[:, b * NT, :, :])
                xts.append(xt)
                if b < 3:
                    # the junk columns [65:129) are consumed as stationary
                    # weights by the PV matmuls (their PSUM rows are never
                    # read) — zero them once per buffer so leftover SBUF bits
                    # can't be NaN/Inf patterns (v1 pool has 3 buffers and
                    # later writes never touch the junk range).
                    nc.gpsimd.memset(v1[:, :, 65:129], 0.0)
                nc.vector.tensor_copy(out=v1[:, :, DH:DH + 1], in_=ones16)
                nc.vector.tensor_copy(out=v1[:, :, 97:98], in_=ones16)
                yield
                for tt in range(NT):
                    if tt + 1 < NT:
                        xt = xin.tile([128, CCH, 512], BF16, tag="x",
                                      name=f"x{b}_{tt + 1}")
                        nc.sync.dma_start(
                            out=xt, in_=xp[:, b * NT + tt + 1, :, :])
                        xts.append(xt)
                    t0 = tt * 512
                    xt = xts[tt]
                    for w_sb, b_sb, dest in ((wq_sb, bq_sb, qT),
                                             (wk_sb, bk_sb, kT)):
                        ps = aux_ps.tile([128, 512], F32, tag="aux")
                        for ci in range(CCH):
                            nc.tensor.matmul(out=ps, lhsT=w_sb[:, ci, :],
                                             rhs=xt[:, ci, :],
                                             start=(ci == 0),
                                             stop=(ci == CCH - 1))
                            if ci % 2 == 1:
                                yield
                        nc.vector.tensor_scalar_add(out=dest[:, t0:t0 + 512],
                                                    in0=ps, scalar1=b_sb)
                        yield
                    ps = aux_ps.tile([128, 512], F32, tag="aux")
                    for ci in range(CCH):
                        nc.tensor.matmul(out=ps, lhsT=wv_sb[:, ci, :],
                                         rhs=xt[:, ci, :],
                                         start=(ci == 0), stop=(ci == CCH - 1))
                        if ci % 2 == 1:
                            yield
                    vt = vtmp_pool.tile([128, 512], F32)
                    nc.vector.tensor_scalar_add(out=vt, in0=ps, scalar1=bv_sb)
                    yield
                    for s in range(4):
                        tp = aux_ps.tile([128, 512], F32, tag="aux")
                        nc.tensor.transpose(out=tp[:, 0:128],
                                            in_=vt[:, s * 128:(s + 1) * 128],
                                            identity=ident)
                        kt = tt * 4 + s
                        sl = v1[:, kt, :]
                        dst = bass.AP(tensor=sl.tensor, offset=sl.offset,
                                      ap=[list(sl.ap[0]), [129, 2], [1, DH]])
                        nc.vector.tensor_copy(
                            out=dst,
                            in_=tp[:, 0:128].rearrange("p (g x) -> p g x", g=2))
                        yield

            def gen_outproj(b, oT2, ts_lo, ts_hi, on_scalar=False):
                """Output-projection units for batch b, t-tiles [ts_lo, ts_hi)
                (needs the corresponding oT2 columns written). on_scalar moves
                the PSUM evacuation to ScalarE (for the epilogue, where the
                Activation engine is idle but DVE still has attention work)."""
                for ts in range(ts_lo, ts_hi):
                    ot = outsb.tile([128, C], F32)
                    for n in range(2):
                        n0 = n * 512
                        ops = aux_ps.tile([128, 512], F32, tag="aux")
                        nc.tensor.matmul(
                            out=ops,
                            lhsT=oT2[:, ts * 128:(ts + 1) * 128],
                            rhs=wo_sb[:, n0:n0 + 512],
                            start=True, stop=True)
                        if on_scalar:
                            nc.scalar.activation(out=ot[:, n0:n0 + 512],
                                                 in_=ops, func=AF.Copy)
                        else:
                            nc.vector.tensor_copy(out=ot[:, n0:n0 + 512],
                                                  in_=ops)
                        yield
                    nc.sync.dma_start(
                        out=y[b * T + ts * 128:b * T + (ts + 1) * 128, :],
                        in_=ot)

            class Injector:
                """Two queues: `pri` (QKV of the next batch — must fully issue
                before that batch's attention starts) and `bg` (out-projection
                chunks — can spill across batch boundaries)."""

                def __init__(self):
                    self.pri = []
                    self.bg = []

                def add(self, g):
                    self.pri.append(g)

                def add_bg(self, g):
                    self.bg.append(g)

                def step(self, n):
                    while n > 0:
                        q = self.pri if self.pri else self.bg
                        if not q:
                            return
                        try:
                            next(q[0])
                            n -= 1
                        except StopIteration:
                            q.pop(0)

                def drain_pri(self):
                    while self.pri:
                        try:
                            next(self.pri[0])
                        except StopIteration:
                            self.pri.pop(0)

                def drain_all(self):
                    self.drain_pri()
                    while self.bg:
                        try:
                            next(self.bg[0])
                        except StopIteration:
                            self.bg.pop(0)

            def attention(b, inj):
                qT, kT, v1 = qkv_tiles[b]
                oT2 = osb.tile([128, T], BF16, tag="o2", name=f"o2_{b}")
                lh = [(0, DH + 1), (DH + 1, 193)]
                for qt in range(NT):
                    q0 = qt * 512
                    pv0 = pv_ps.tile([128, 512], F32, tag="pv")
                    pv1 = pv_ps.tile([128, 512], F32, tag="pv")
                    pvs = [pv0[0:DH + 1, :], pv1[:, :]]
                    ets = []
                    LA = 2  # PV lookahead: PV(kt-LA) issues with S(kt)
                    for kt in range(NK):
                        s2 = s2_ps.tile([128, 1024], F32, tag="s2")
                        for h in range(HPC):
                            hs = h * DH
                            nc.tensor.matmul(
                                out=s2[:, h * 512:(h + 1) * 512],
                                lhsT=kT[hs:hs + DH, kt * 128:(kt + 1) * 128],
                                rhs=qT[hs:hs + DH, q0:q0 + 512],
                                start=True, stop=True)
                        if kt >= LA:
                            etp = ets[kt - LA]
                            for h in range(HPC):
                                nc.tensor.matmul(
                                    out=pvs[h],
                                    lhsT=v1[:, kt - LA, lh[h][0]:lh[h][1]],
                                    rhs=etp[:, h * 512:(h + 1) * 512],
                                    start=(kt - LA == 0), stop=False)
                        et = esb.tile([128, 1024], BF16)
                        nc.scalar.activation(out=et, in_=s2,
                                             func=AF.Exp, scale=0.125)
                        ets.append(et)
                        inj.step(1 if kt % 2 else 2)
                    for kt in range(NK - LA, NK):
                        for h in range(HPC):
                            nc.tensor.matmul(
                                out=pvs[h],
                                lhsT=v1[:, kt, lh[h][0]:lh[h][1]],
                                rhs=ets[kt][:, h * 512:(h + 1) * 512],
                                start=False, stop=(kt == NK - 1))
                    # evacuate each PV accumulator with one SBUF copy (frees
                    # the PSUM bank for the next qt immediately), then
                    # normalize from the copy.
                    # h0: num@[0:64], Z@[64]; h1: num@[64:128], Z@[32]
                    for h, pv, zrow, o_lo, o_hi in (
                            (0, pv0, DH, 0, DH),
                            (1, pv1, 32, DH, 128)):
                        z = rsb.tile([1, 512], F32, tag="z")
                        nc.vector.tensor_copy(out=z, in_=pv[zrow:zrow + 1, :])
                        r = rsb.tile([1, 512], F32, tag="r")
                        nc.vector.reciprocal_approx_fast(out=r, in_=z)
                        rbc = rsb.tile([128, 512], F32, tag="rbc")
                        nc.gpsimd.partition_broadcast(rbc[0:o_hi, :], r)
                        nc.vector.tensor_mul(out=oT2[o_lo:o_hi, q0:q0 + 512],
                                             in0=pv[o_lo:o_hi, :],
                                             in1=rbc[o_lo:o_hi, :])
                    # this qt's out-projection columns are now computable
                    inj.add_bg(gen_outproj(b, oT2, qt * 4, qt * 4 + 4,
                                            on_scalar=(b == B - 1 and qt == 3)))
                    inj.step(2)
                return oT2

            # ---- pipelined schedule ----
            inj = Injector()
            inj.add(gen_qkv(0))
            inj.step(1)            # fires the x(0,0) DMA right after wq's
            load_rest_of_weights()
            inj.drain_pri()
            for b in range(B):
                if b + 1 < B:
                    inj.add(gen_qkv(b + 1))
                attention(b, inj)
                inj.drain_pri()
            inj.drain_all()

    nc.compile()
    return nc


def make_in_maps(x, W_qkv, b_qkv, W_out, b_out):
    import ml_dtypes
    bf = ml_dtypes.bfloat16
    # x pre-tiled to the exact SBUF layout: xp[p, tile, ci, c] = x[tile*512+c, ci*128+p]
    xp = np.ascontiguousarray(
        x.reshape(B * NT, 512, CCH, 128).transpose(3, 0, 2, 1).astype(bf))
    in_maps = []
    for c in range(NCORES):
        r0 = c * D2
        def wshuf(wslice):
            # [D2, C] weight rows -> lhsT chunks [128 p, CCH, D2]
            return np.ascontiguousarray(
                wslice.T.reshape(CCH, 128, D2).transpose(1, 0, 2).astype(bf))
        wqc = wshuf(W_qkv[r0:r0 + D2, :])
        wkc = wshuf(W_qkv[C + r0:C + r0 + D2, :])
        wvc = wshuf(W_qkv[2 * C + r0:2 * C + r0 + D2, :])
        bqc = np.ascontiguousarray(b_qkv[r0:r0 + D2].reshape(D2, 1))
        bkc = np.ascontiguousarray(b_qkv[C + r0:C + r0 + D2].reshape(D2, 1))
        bvc = np.ascontiguousarray(b_qkv[2 * C + r0:2 * C + r0 + D2].reshape(D2, 1))
        woc = np.ascontiguousarray(W_out[:, r0:r0 + D2].T.astype(bf))
        in_maps.append({
            "xp": xp, "wq": wqc, "wk": wkc, "wv": wvc,
            "bq": bqc, "bk": bkc, "bv": bvc, "wo": woc,
        })
    return in_maps


def run(x, W_qkv, b_qkv, W_out, b_out, trace=False):
    if "nc" not in _NC_CACHE:
        _NC_CACHE["nc"] = build_nc()
    nc = _NC_CACHE["nc"]
    in_maps = make_in_maps(
        np.asarray(x, dtype=np.float32), np.asarray(W_qkv, dtype=np.float32),
        np.asarray(b_qkv, dtype=np.float32), np.asarray(W_out, dtype=np.float32),
        np.asarray(b_out, dtype=np.float32))
    res = run_bass_kernel_spmd(nc, in_maps, core_ids=list(range(NCORES)),
                               trace=trace)
    acc = np.zeros((BT, C), dtype=np.float64)
    for c in range(NCORES):
        acc += res.results[c]["y"]
    acc += np.asarray(b_out, dtype=np.float64)[None, :]
    out = acc.astype(np.float32).reshape(B, T, C)
    return out, res


def kernel(x, W_qkv, b_qkv, W_out, b_out):
    out, _ = run(x, W_qkv, b_qkv, W_out, b_out, trace=False)
    return out
